# revision 1
# baseline (speedup 1.0000x reference)
"""Cross-entropy loss with gaussian-smoothed labels on 8 Trainium2 NeuronCores.

Math: reference scatters DECAYS[dist] at clip(t +/- dist) for dist 3..0, nearer
distances overwriting farther.  Because the clipped (colliding) writes at a
boundary always resolve to DECAYS[|j - t|], the smoothed one-hot is exactly
    w(j) = DECAYS[|j - t|]  if |j - t| <= 3 else 0,   j in [0, C).
With logp = pred - lse(pred):
    loss = mean_f [ Wsum_f * lse_f - sum_k w_k * pred[f, idx_k] ]
Each core handles 8192 frames (data-parallel over B*T = 65536 frames):
  - streams its [8192, 722] pred shard through SBUF, one ACT Exp pass per
    [128, 722] tile with accum_out giving the softmax denominator,
  - gathers the 8-wide class windows with one indirect DMA per 128-frame
    tile (HW consumes one offset per dest partition, copying the dest row
    as contiguous elements from offset frame*722 + clip(t-3, 0, 714)),
  - builds the 8-wide weight vectors on-chip from the targets,
  - reduces to per-partition partials; host sums 8x128 values / N.
"""

import math
from contextlib import ExitStack

import numpy as np

import concourse.bass as bass
import concourse.bacc as bacc
from concourse import mybir
from concourse.bass_utils import run_bass_kernel_spmd
from concourse.tile import TileContext

C = 722           # num classes
P = 128           # partitions
N_CORES = 8
FRAMES = 16 * 4096
FPC = FRAMES // N_CORES   # 8192 frames per core
NT = FPC // P             # 64 tiles of 128 frames
WIN = 8                   # gathered window width
SMAX = C - WIN            # 714: max window start, keeps window inside the frame
G = 1                     # tiles per pred DMA group
PRED_BUFS = 6
ESC_BUFS = 4
SPLIT_DMA = False
LN2 = float(math.log(2.0))

f32 = mybir.dt.float32
i32 = mybir.dt.int32
Act = mybir.ActivationFunctionType
Alu = mybir.AluOpType

_CACHE: dict = {}


def _build_module() -> bass.Bass:
    nc = bacc.Bacc(None, target_bir_lowering=False)
    pred = nc.declare_dram_parameter("pred", [FPC, C], f32, isOutput=False)
    tgt = nc.declare_dram_parameter("tgt_t", [P, NT], i32, isOutput=False)
    out = nc.declare_dram_parameter("out", [P, 1], f32, isOutput=True)

    with TileContext(nc) as tc, ExitStack() as ctx:
        const = ctx.enter_context(tc.tile_pool(name="const", bufs=1))
        pred_pool = ctx.enter_context(tc.tile_pool(name="predp", bufs=PRED_BUFS))
        esc_pool = ctx.enter_context(tc.tile_pool(name="escp", bufs=ESC_BUFS))

        # --- setup: targets -> window starts, gather indices, weights ---
        tgt_sb = const.tile([P, NT], i32)
        nc.sync.dma_start(out=tgt_sb[:], in_=tgt[:])

        # s = clip(t - 3, 0, 714)
        s_sb = const.tile([P, NT], i32)
        nc.vector.tensor_scalar(
            out=s_sb[:], in0=tgt_sb[:], scalar1=3, scalar2=0,
            op0=Alu.subtract, op1=Alu.max)
        nc.vector.tensor_scalar(
            out=s_sb[:], in0=s_sb[:], scalar1=SMAX, scalar2=None, op0=Alu.min)

        # element index of window start: frame*C + s, frame = n*128 + p.
        # iota steps are int16-limited, so frame*C is computed in f32
        # (exact: max value 5.9e6 < 2^24).
        iota0 = const.tile([P, NT], i32)
        nc.gpsimd.iota(iota0[:], pattern=[[P, NT]], channel_multiplier=1)
        fidx = const.tile([P, NT], f32)
        nc.vector.tensor_copy(out=fidx[:], in_=iota0[:])
        s_f = const.tile([P, NT], f32)
        nc.vector.tensor_copy(out=s_f[:], in_=s_sb[:])
        nc.vector.tensor_scalar(
            out=fidx[:], in0=fidx[:], scalar1=float(C), scalar2=None,
            op0=Alu.mult)
        nc.vector.tensor_tensor(out=fidx[:], in0=fidx[:], in1=s_f[:], op=Alu.add)
        idx = const.tile([P, NT], i32)
        nc.vector.tensor_copy(out=idx[:], in_=fidx[:])

        # delta_neg = s - t in [-7, 0], as f32
        dneg_i = const.tile([P, NT], i32)
        nc.vector.tensor_tensor(
            out=dneg_i[:], in0=s_sb[:], in1=tgt_sb[:], op=Alu.subtract)
        dneg_f = const.tile([P, NT], f32)
        nc.vector.tensor_copy(out=dneg_f[:], in_=dneg_i[:])

        # window weights: w[i] = exp(-2^|i-delta|/4) * (|i-delta| <= 3)
        iota_w_i = const.tile([P, NT, WIN], i32)
        nc.gpsimd.iota(
            iota_w_i[:], pattern=[[0, NT], [1, WIN]], channel_multiplier=0)
        iota_w = const.tile([P, NT, WIN], f32)
        nc.vector.tensor_copy(out=iota_w[:], in_=iota_w_i[:])

        dsum = const.tile([P, NT, WIN], f32)
        nc.vector.tensor_tensor(
            out=dsum[:], in0=iota_w[:],
            in1=dneg_f[:].to_broadcast([P, NT, WIN]), op=Alu.add)
        d_abs = const.tile([P, NT, WIN], f32)
        nc.scalar.activation(out=d_abs[:], in_=dsum[:], func=Act.Abs)
        p2 = const.tile([P, NT, WIN], f32)
        nc.scalar.activation(out=p2[:], in_=d_abs[:], func=Act.Exp, scale=LN2)
        w_un = const.tile([P, NT, WIN], f32)
        nc.scalar.activation(out=w_un[:], in_=p2[:], func=Act.Exp, scale=-0.25)
        mask = const.tile([P, NT, WIN], f32)
        nc.vector.tensor_scalar(
            out=mask[:], in0=d_abs[:], scalar1=3.5, scalar2=None, op0=Alu.is_le)
        w_all = const.tile([P, NT, WIN], f32)
        nc.vector.tensor_tensor(
            out=w_all[:], in0=w_un[:], in1=mask[:], op=Alu.mult)
        wsum = const.tile([P, NT], f32)
        nc.vector.reduce_sum(
            out=wsum[:], in_=w_all[:], axis=mybir.AxisListType.X)

        # Window gathers: HW indirect DMA consumes ONE offset per dest
        # partition and copies the dest row as contiguous elements, so each
        # 128-frame tile needs its own gather (idx[:, n] -> win[:, n*8:+8]).
        # Dest must be a clean 2D AP.
        win_all = const.tile([P, NT * WIN], f32)
        for n in range(NT):
            nc.gpsimd.indirect_dma_start(
                out=win_all[:, n * WIN:(n + 1) * WIN], out_offset=None,
                in_=pred[:],
                in_offset=bass.IndirectOffsetOnAxis(
                    ap=idx[:, n:n + 1], axis=1),
            )

        # --- main loop: stream pred, Exp with row-sum accumulation ---
        S_all = const.tile([P, NT], f32)
        pred_view = pred[:].rearrange("(n p) c -> p n c", p=P)
        for g0 in range(0, NT, G):
            shape = [P, C] if G == 1 else [P, G, C]
            ptile = pred_pool.tile(shape, f32, tag="ptile")
            eng = nc.sync if (not SPLIT_DMA or (g0 // G) % 2 == 0) else nc.scalar
            if G == 1:
                eng.dma_start(out=ptile[:], in_=pred_view[:, g0, :])
            else:
                eng.dma_start(out=ptile[:], in_=pred_view[:, g0:g0 + G, :])
            for j in range(G):
                n = g0 + j
                esc = esc_pool.tile([P, C], f32, tag="esc")
                src_ap = ptile[:] if G == 1 else ptile[:, j, :]
                nc.scalar.activation(
                    out=esc[:], in_=src_ap, func=Act.Exp,
                    accum_out=S_all[:, n:n + 1])

        # --- epilogue: two fused multiply-reduces and a partition sum ---
        lse = const.tile([P, NT], f32)
        nc.scalar.activation(out=lse[:], in_=S_all[:], func=Act.Ln)
        o1 = const.tile([P, NT], f32)
        acc1 = const.tile([P, 1], f32)
        nc.vector.tensor_mul(out=o1[:], in0=lse[:], in1=wsum[:])
        nc.vector.reduce_sum(out=acc1[:], in_=o1[:], axis=mybir.AxisListType.X)
        o2 = const.tile([P, NT, WIN], f32)
        acc2 = const.tile([P, 1], f32)
        nc.vector.tensor_mul(
            out=o2[:], in0=w_all[:],
            in1=win_all[:].rearrange("p (n w) -> p n w", w=WIN))
        nc.vector.reduce_sum(out=acc2[:], in_=o2[:], axis=mybir.AxisListType.XY)
        # per-partition partials; host sums the 128 values per core
        res = const.tile([P, 1], f32)
        nc.vector.tensor_sub(out=res[:], in0=acc1[:], in1=acc2[:])
        nc.sync.dma_start(out=out[:], in_=res[:])

    nc.finalize()
    return nc


def _prep_inputs(pred: np.ndarray, target: np.ndarray):
    """Shard full inputs into per-core input maps."""
    pred_flat = np.ascontiguousarray(
        np.asarray(pred, dtype=np.float32).reshape(FRAMES, C))
    tgt_flat = np.asarray(target).astype(np.int32).reshape(FRAMES)
    in_maps = []
    for k in range(N_CORES):
        p_shard = pred_flat[k * FPC:(k + 1) * FPC]
        t_shard = tgt_flat[k * FPC:(k + 1) * FPC]
        # device sees targets as [128, 64]: [p, n] = frame n*128 + p
        tgt_t = np.ascontiguousarray(t_shard.reshape(NT, P).T)
        in_maps.append({"pred": p_shard, "tgt_t": tgt_t})
    return in_maps


def kernel(pred: np.ndarray, target: np.ndarray, **_unused) -> np.ndarray:
    if "nc" not in _CACHE:
        _CACHE["nc"] = _build_module()
    nc = _CACHE["nc"]
    in_maps = _prep_inputs(pred, target)
    res = run_bass_kernel_spmd(nc, in_maps, core_ids=list(range(N_CORES)))
    total = sum(float(r["out"].sum(dtype=np.float64)) for r in res.results)
    return np.float32(total / FRAMES)

